# revision 3
# baseline (speedup 1.0000x reference)
"""2-layer GAT on 8 Trainium2 NeuronCores (axon).

Strategy (dst-sharded, 3 launches, host relays shards between launches):
  k1: per-core dense projection  [h1|as1|ad1] = x_shard @ [W1|vs1|vd1]
  k2: per-core layer-1 edge phase over this core's dst tiles:
      dma_gather of [h1|as1] rows by src (low/high int16 table split),
      alpha_d expansion via run-range one-hot matmul, edge softmax
      (no max-subtraction needed: exp args are bounded), one-hot matmul
      segment-sum with the softmax denominator riding as 8 extra rhs
      columns, then ELU + dense h2 = h_elu @ [W2|0|vs2|vd2].
  k3: same edge phase for layer 2 (heads=1, 40 classes) -> output.

Self-contained: hardcodes problem shapes from the spec.
"""
import sys
sys.path.insert(0, "/opt/trn_rl_repo")
import inspect
import textwrap
from contextlib import ExitStack

import numpy as np
import ml_dtypes

import concourse.bass as bass
import concourse.bacc as bacc
import concourse.tile as tile
import concourse.mybir as mybir
from concourse.masks import make_identity

bf16 = mybir.dt.bfloat16
f32 = mybir.dt.float32
i16 = mybir.dt.int16
AOT = mybir.AluOpType

N_NODES = 50000
N_EDGES = 800000
IN_DIM = 256
FPH = 32
HEADS = 8
HIDDEN = 256
NUM_CLASSES = 40
NEG_SLOPE = 0.2

NCORES = 8
NP = 50176              # padded nodes = 8 * 49 * 128
TPC = 49                # dst tiles per core
NT = NCORES * TPC       # 392 tiles
LOWN = 32768            # nodes with permuted id < LOWN gather from table part 1

G1W = 384               # g1 table row stride (elems bf16; 768B)
G1E = 264               # g1 gather elem count ([h(256)|as(8)])
G2W = 128               # g2 table row stride (256B)
G2E = 43                # g2 gather elem ([h2(40)|1|as2|ad2])

# dma_gather without the elem%256 assert (stride still must be %256B)
_src = textwrap.dedent(inspect.getsource(bass.BassGpSimd.dma_gather))
_src = _src.replace(
    "assert (\n        elem_size_bytes > 0 and elem_size_bytes % 256 == 0\n    )  # transpose restriction",
    "assert elem_size_bytes > 0")
assert "% 256 == 0" not in _src
_ns = dict(vars(bass))
exec(_src, _ns)
_dma_gather = _ns["dma_gather"]


# ---------------------------------------------------------------- host prep --
def _pack(edge_index):
    """Permute nodes into 392 dst-tiles with per-tile low/high edge capacity.

    Returns permutation arrays and per-tile slot tables.
    """
    src0 = edge_index[0].astype(np.int64)
    dst0 = edge_index[1].astype(np.int64)
    rng = np.random.default_rng(12345)

    # fixed random low/high pool split (independent of tile packing)
    pool_perm = rng.permutation(NP)
    is_low = np.zeros(NP, bool)
    is_low[pool_perm[:LOWN]] = True

    deg_lo = np.bincount(dst0[is_low[src0]], minlength=NP)
    deg_hi = np.bincount(dst0[~is_low[src0]], minlength=NP)

    # bin-pack: low-pool nodes into tiles 0..255, high-pool into 256..391,
    # constraint: per-tile sum(deg_lo) <= KLO*128 and sum(deg_hi) <= KHI*128
    def pack_pool(nodes, ntiles, cap_lo, cap_hi):
        order = nodes[np.argsort(-(deg_lo[nodes] + deg_hi[nodes]))]
        rem_lo = np.full(ntiles, cap_lo, np.int64)
        rem_hi = np.full(ntiles, cap_hi, np.int64)
        cnt = np.zeros(ntiles, np.int64)
        assign = np.empty(len(order), np.int64)
        for k, n in enumerate(order):
            # feasible tiles with space for the node
            ok = (rem_lo >= deg_lo[n]) & (rem_hi >= deg_hi[n]) & (cnt < 128)
            cand = np.nonzero(ok)[0]
            assert len(cand), "bin packing failed; bump KLO/KHI"
            t = cand[np.argmax(np.minimum(rem_lo[cand], rem_hi[cand] * 2))]
            assign[k] = t
            rem_lo[t] -= deg_lo[n]
            rem_hi[t] -= deg_hi[n]
            cnt[t] += 1
        return order, assign

    # capacities: choose smallest K that packs with slack
    elo = int(deg_lo.sum())
    ehi = int(deg_hi.sum())
    KLO = max(2, int(np.ceil(elo / NT / 128 * 1.12)))
    KHI = max(1, int(np.ceil(ehi / NT / 128 * 1.12)))

    low_nodes = np.nonzero(is_low)[0]
    high_nodes = np.nonzero(~is_low)[0]
    ntiles_lo = LOWN // 128          # 256
    ntiles_hi = (NP - LOWN) // 128   # 136
    o1, a1 = pack_pool(low_nodes, ntiles_lo, KLO * 128, KHI * 128)
    o2, a2 = pack_pool(high_nodes, ntiles_hi, KLO * 128, KHI * 128)

    # permuted id: tile t gets its nodes in arbitrary order
    perm = np.empty(NP, np.int64)   # old id -> new id
    fill = np.zeros(NT, np.int64)
    for nodes, assign, toff in ((o1, a1, 0), (o2, a2, ntiles_lo)):
        t = assign + toff
        # stable order: iterate
        for n, ti in zip(nodes, t):
            perm[n] = ti * 128 + fill[ti]
            fill[ti] += 1
    assert (fill <= 128).all()
    # leftover slots (tiles not full) get... all nodes assigned (cnt<128 enforced)
    assert fill.sum() == NP

    inv = np.empty(NP, np.int64)    # new id -> old id
    inv[perm] = np.arange(NP)

    # interleave tiles across cores so each core gets a mix of low/high tiles
    # core c owns global tiles c, c+8, c+16, ...  -> local tile index j = t//8
    src_p = perm[src0]
    dst_p = perm[dst0]
    tile_of_edge = dst_p // 128
    core_of_edge = tile_of_edge % NCORES
    ltile_of_edge = tile_of_edge // NCORES

    KT = KLO + KHI
    idx_lo = np.full((NCORES, TPC, KLO * 128), -1, np.int64)
    idx_hi = np.full((NCORES, TPC, KHI * 128), -1, np.int64)
    ldst = np.full((NCORES, TPC, KT * 128), 255, np.int64)
    # per (core, tile, dst-local, region): run [start, end) in region slots
    run_lo = np.zeros((NCORES, TPC, 128, 2), np.int64)
    run_hi = np.zeros((NCORES, TPC, 128, 2), np.int64)

    e_low = is_low[src0]
    for c in range(NCORES):
        for j in range(TPC):
            m = (core_of_edge == c) & (ltile_of_edge == j)
            for lowreg in (True, False):
                mm = m & (e_low == lowreg)
                s = src_p[mm]
                d = dst_p[mm] % 128
                o = np.argsort(d, kind="stable")
                s, d = s[o], d[o]
                n = len(s)
                if lowreg:
                    assert n <= KLO * 128, (n, KLO * 128)
                    idx_lo[c, j, :n] = s
                    ldst[c, j, :n] = d
                    st = np.searchsorted(d, np.arange(128), "left")
                    en = np.searchsorted(d, np.arange(128), "right")
                    run_lo[c, j, :, 0] = st
                    run_lo[c, j, :, 1] = en
                else:
                    assert n <= KHI * 128, (n, KHI * 128)
                    idx_hi[c, j, :n] = s - LOWN
                    ldst[c, j, KLO * 128:KLO * 128 + n] = d
                    st = np.searchsorted(d, np.arange(128), "left")
                    en = np.searchsorted(d, np.arange(128), "right")
                    run_hi[c, j, :, 0] = st
                    run_hi[c, j, :, 1] = en
    # -1 indices wedge the SWDGE gather rings; point pad slots at row 0
    # (ldst=255 makes the one-hot kill their contribution).
    idx_lo = np.maximum(idx_lo, 0)
    idx_hi = np.maximum(idx_hi, 0)
    return dict(perm=perm, inv=inv, KLO=KLO, KHI=KHI, KT=KT,
                idx_lo=idx_lo, idx_hi=idx_hi, ldst=ldst,
                run_lo=run_lo, run_hi=run_hi)


def _wrap_idx(arr):
    """[..., n] int -> wrapped int16 [..., 128, n//16] (replicated x8)."""
    n = arr.shape[-1]
    a = arr.astype(np.int16).reshape(*arr.shape[:-1], n // 16, 16)
    a = np.swapaxes(a, -1, -2)                       # [..., 16, n//16]
    return np.tile(a, (1,) * (arr.ndim - 1) + (8, 1))


def _clip_runs(run, K):
    """[..., 128, 2] region runs -> per-chunk clipped [..., K, 128, 2] f32."""
    st = run[..., None, :, 0]   # [..., 1, 128]
    en = run[..., None, :, 1]
    c = np.arange(K)[:, None] * 128
    lo = np.clip(st - c, 0, 128)
    hi = np.clip(en - c, 0, 128)
    return np.stack([lo, hi], axis=-1).astype(np.float32)  # [..., K, 128, 2]


# ------------------------------------------------------------ bass builders --
def build_k1():
    nc = bacc.Bacc("TRN2", target_bir_lowering=False, debug=False)
    xT = nc.dram_tensor("xT", [IN_DIM, TPC * 128], bf16, kind="ExternalInput")
    W = nc.dram_tensor("W", [IN_DIM, 272], bf16, kind="ExternalInput")
    g1 = nc.dram_tensor("g1", [TPC * 128, G1W], bf16, kind="ExternalOutput")

    with tile.TileContext(nc) as tc, ExitStack() as ctx:
        sb = ctx.enter_context(tc.tile_pool(name="sb", bufs=3))
        cpool = ctx.enter_context(tc.tile_pool(name="c", bufs=1))
        ps = ctx.enter_context(tc.tile_pool(name="ps", bufs=2, space="PSUM"))
        Wsb = cpool.tile([128, 2 * 272], bf16)
        nc.sync.dma_start(Wsb[:].rearrange("p (a n) -> p a n", a=2), W[:, :].rearrange("(a p) n -> p a n", p=128))
        for t in range(TPC):
            xt = sb.tile([128, 2 * 128], bf16, tag="xt")
            nc.sync.dma_start(
                xt[:].rearrange("p (a n) -> p a n", a=2),
                xT[:, t * 128:(t + 1) * 128].rearrange("(a p) n -> p a n", p=128))
            acc = ps.tile([128, 272], f32, space="PSUM", tag="acc")
            for k in range(2):
                nc.tensor.matmul(acc[:], lhsT=xt[:, k * 128:(k + 1) * 128],
                                 rhs=Wsb[:, k * 272:(k + 1) * 272],
                                 start=(k == 0), stop=(k == 1))
            ob = sb.tile([128, 272], bf16, tag="ob")
            nc.vector.tensor_copy(ob[:], acc[:])
            nc.sync.dma_start(g1[t * 128:(t + 1) * 128, 0:272], ob[:])
    nc.compile()
    return nc


def _edge_layer(nc, tc, ctx, *, tab, tabw, elem, nlow_tab, KLO, KHI,
                idx_lo, idx_hi, ldst, rnglo, rnghi, adt, adw,
                flush_fn, heads, ex_slot):
    """Shared edge phase. flush_fn(t, agg_psum) consumes the per-tile psum.

    tab: dram gather table [NP, tabw]; rows < nlow_tab from part 1.
    adt: sbuf [128, TPC*adw] alpha_d per tile.
    heads: 8 (ex in g cols 256:264) or 1 (ex folded into one-hot).
    """
    KT = KLO + KHI
    sb = ctx.enter_context(tc.tile_pool(name="esb", bufs=3))
    cpool = ctx.enter_context(tc.tile_pool(name="ec", bufs=1))
    opool = ctx.enter_context(tc.tile_pool(name="eo", bufs=3))
    ps_agg = ctx.enter_context(tc.tile_pool(name="pagg", bufs=2, space="PSUM"))
    ps_ad = ctx.enter_context(tc.tile_pool(name="pad", bufs=2, space="PSUM"))

    iota32 = cpool.tile([128, 128], mybir.dt.int32)
    nc.gpsimd.iota(iota32[:], pattern=[[1, 128]], base=0, channel_multiplier=0)
    iota_bf = cpool.tile([128, 128], bf16)
    nc.vector.tensor_copy(iota_bf[:], iota32[:])

    # zero the gather buffers once (bufs=3 rotate)
    gz = []
    for i in range(3):
        gt = sb.tile([128, KT * elem], bf16, tag="g")
        nc.vector.memset(gt[:], 0.0)
        gz.append(gt)

    for t in range(TPC):
        gt = sb.tile([128, KT * elem], bf16, tag="g")
        glo = gt[:].rearrange("p (c x) -> p c x", x=elem)[:, 0:KLO, :]
        ghi = gt[:].rearrange("p (c x) -> p c x", x=elem)[:, KLO:KT, :]
        _dma_gather(nc.gpsimd, glo, tab[0:nlow_tab, 0:elem],
                    idx_lo[:, t * (KLO * 8):(t + 1) * (KLO * 8)],
                    KLO * 128, KLO * 128, elem, elem_step=tabw,
                    single_packet=False, queue_num=(2 * t) % 4)
        _dma_gather(nc.gpsimd, ghi, tab[nlow_tab:NP, 0:elem],
                    idx_hi[:, t * (KHI * 8):(t + 1) * (KHI * 8)],
                    KHI * 128, KHI * 128, elem, elem_step=tabw,
                    single_packet=False, queue_num=(2 * t + 1) % 4)

        # O2 regions (dst-on-partitions one-hot, via run ranges) + adpe matmuls
        adpe = ps_ad.tile([128, KT * heads], f32, space="PSUM", tag="adpe")
        o2 = opool.tile([128, KT * 128], bf16, tag="o2")
        for reg, K, coff, rng_t in ((0, KLO, 0, rnglo), (1, KHI, KLO, rnghi)):
            for c in range(K):
                cc = coff + c
                rcol = rng_t[:, (t * K + c) * 2:(t * K + c) * 2 + 2]
                b = opool.tile([128, 128], bf16, tag="o2b")
                nc.vector.tensor_scalar(
                    out=b[:], in0=iota_bf[:], scalar1=rcol[:, 1:2],
                    scalar2=None, op0=AOT.is_lt)
                nc.vector.scalar_tensor_tensor(
                    out=o2[:, cc * 128:(cc + 1) * 128], in0=iota_bf[:],
                    scalar=rcol[:, 0:1], in1=b[:],
                    op0=AOT.is_ge, op1=AOT.mult)
                nc.tensor.matmul(
                    adpe[:, cc * heads:(cc + 1) * heads],
                    lhsT=o2[:, cc * 128:(cc + 1) * 128],
                    rhs=adt[:, t * adw:t * adw + heads],
                    start=True, stop=True)

        # t = as + ad ; lrelu ; exp
        g3 = gt[:].rearrange("p (c x) -> p c x", x=elem)
        asv = g3[:, :, ex_slot:ex_slot + heads]
        tv = sb.tile([128, KT * heads], f32, tag="tv")
        nc.vector.tensor_tensor(out=tv[:], in0=adpe[:], in1=asv, op=AOT.add)
        nc.vector.scalar_tensor_tensor(out=tv[:], in0=tv[:], scalar=NEG_SLOPE,
                                       in1=tv[:], op0=AOT.mult, op1=AOT.max)
        if heads == 8:
            # exp back into the gathered as-columns (strided), then msg mult
            nc.scalar.activation(asv, tv[:], mybir.ActivationFunctionType.Exp)
            hv = g3[:, :, 0:256].rearrange("p c (i j) -> p c i j", j=FPH)
            exv = g3[:, :, 256:264].to_broadcast([128, KT, 8, FPH])
            nc.vector.tensor_tensor(out=hv, in0=hv, in1=exv, op=AOT.mult)
        else:
            ex2 = sb.tile([128, KT], f32, tag="ex2")
            nc.scalar.activation(ex2[:], tv[:], mybir.ActivationFunctionType.Exp)

        agg = ps_agg.tile([128, elem if heads == 8 else 41], f32,
                          space="PSUM", tag="agg")
        for c in range(KT):
            oh = opool.tile([128, 128], bf16, tag="oh")
            if heads == 8:
                nc.vector.tensor_scalar(
                    out=oh[:], in0=iota_bf[:],
                    scalar1=ldst[:, t * KT + c:t * KT + c + 1],
                    scalar2=None, op0=AOT.is_equal)
                rhs = g3[:, c, :]
            else:
                nc.vector.tensor_scalar(
                    out=oh[:], in0=iota_bf[:],
                    scalar1=ldst[:, t * KT + c:t * KT + c + 1],
                    scalar2=ex2[:, c:c + 1],
                    op0=AOT.is_equal, op1=AOT.mult)
                rhs = g3[:, c, 0:41]
            nc.tensor.matmul(agg[:], lhsT=oh[:], rhs=rhs,
                             start=(c == 0), stop=(c == KT - 1))
        flush_fn(t, agg)


def build_k2(KLO, KHI):
    KT = KLO + KHI
    nc = bacc.Bacc("TRN2", target_bir_lowering=False, debug=False,
                   num_swdge_queues=4)
    g1tab = nc.dram_tensor("g1tab", [NP, G1W], bf16, kind="ExternalInput")
    ad1 = nc.dram_tensor("ad1", [128, TPC * 8], bf16, kind="ExternalInput")
    ilo = nc.dram_tensor("ilo", [128, TPC * KLO * 8], i16, kind="ExternalInput")
    ihi = nc.dram_tensor("ihi", [128, TPC * KHI * 8], i16, kind="ExternalInput")
    lds = nc.dram_tensor("lds", [128, TPC * KT], f32, kind="ExternalInput")
    rlo = nc.dram_tensor("rlo", [128, TPC * KLO * 2], f32, kind="ExternalInput")
    rhi = nc.dram_tensor("rhi", [128, TPC * KHI * 2], f32, kind="ExternalInput")
    b1r = nc.dram_tensor("b1r", [128, 256], f32, kind="ExternalInput")
    W2 = nc.dram_tensor("W2", [HIDDEN, 48], bf16, kind="ExternalInput")
    g2 = nc.dram_tensor("g2", [TPC * 128, G2W], bf16, kind="ExternalOutput")

    with tile.TileContext(nc) as tc, ExitStack() as ctx:
        cp = ctx.enter_context(tc.tile_pool(name="cp", bufs=1))
        fsb = ctx.enter_context(tc.tile_pool(name="fsb", bufs=2))
        ps_tr = ctx.enter_context(tc.tile_pool(name="ptr", bufs=2, space="PSUM"))
        ps_g2 = ctx.enter_context(tc.tile_pool(name="pg2", bufs=2, space="PSUM"))

        adt = cp.tile([128, TPC * 8], bf16)
        nc.sync.dma_start(adt[:], ad1[:, :])
        ilot = cp.tile([128, TPC * KLO * 8], i16)
        nc.sync.dma_start(ilot[:], ilo[:, :])
        ihit = cp.tile([128, TPC * KHI * 8], i16)
        nc.sync.dma_start(ihit[:], ihi[:, :])
        ldst = cp.tile([128, TPC * KT], f32)
        nc.sync.dma_start(ldst[:], lds[:, :])
        rlot = cp.tile([128, TPC * KLO * 2], f32)
        nc.sync.dma_start(rlot[:], rlo[:, :])
        rhit = cp.tile([128, TPC * KHI * 2], f32)
        nc.sync.dma_start(rhit[:], rhi[:, :])
        b1t = cp.tile([128, 256], f32)
        nc.sync.dma_start(b1t[:], b1r[:, :])
        W2t = cp.tile([128, 2 * 48], bf16)
        nc.sync.dma_start(W2t[:].rearrange("p (a n) -> p a n", a=2), W2[:, :].rearrange("(a p) n -> p a n", p=128))
        ident = cp.tile([128, 128], bf16)
        make_identity(nc, ident[:])

        def flush(t, agg):
            # softmax denominator + normalize + bias + ELU
            rec = fsb.tile([128, 8], f32, tag="rec")
            nc.vector.tensor_scalar_add(rec[:], agg[:, 256:264], 1e-16)
            nc.vector.reciprocal(rec[:], rec[:])
            v = fsb.tile([128, 256], f32, tag="v")
            av = agg[:, 0:256].rearrange("p (i j) -> p i j", j=FPH)
            rv = rec[:].to_broadcast([128, 8, FPH])
            nc.vector.tensor_tensor(out=v[:].rearrange("p (i j) -> p i j", j=FPH),
                                    in0=av, in1=rv, op=AOT.mult)
            nc.vector.tensor_tensor(out=v[:], in0=v[:], in1=b1t[:], op=AOT.add)
            m = fsb.tile([128, 256], f32, tag="m")
            nc.vector.tensor_scalar_min(m[:], v[:], 0.0)
            nc.scalar.activation(m[:], m[:], mybir.ActivationFunctionType.Exp)
            nc.vector.tensor_scalar_add(m[:], m[:], -1.0)
            helu = fsb.tile([128, 256], bf16, tag="helu")
            nc.vector.tensor_tensor(out=helu[:], in0=v[:], in1=m[:], op=AOT.max)
            # transpose for the dense projection
            hT = fsb.tile([128, 2 * 128], bf16, tag="hT")
            for k in range(2):
                tp = ps_tr.tile([128, 128], bf16, space="PSUM", tag="tp")
                nc.tensor.transpose(tp[:], helu[:, k * 128:(k + 1) * 128],
                                    ident[:])
                nc.vector.tensor_copy(hT[:, k * 128:(k + 1) * 128], tp[:])
            g2p = ps_g2.tile([128, 48], f32, space="PSUM", tag="g2p")
            for k in range(2):
                nc.tensor.matmul(g2p[:], lhsT=hT[:, k * 128:(k + 1) * 128],
                                 rhs=W2t[:, k * 48:(k + 1) * 48],
                                 start=(k == 0), stop=(k == 1))
            ob = fsb.tile([128, 48], bf16, tag="ob")
            nc.vector.tensor_copy(ob[:], g2p[:])
            nc.vector.memset(ob[:, 40:41], 1.0)
            nc.sync.dma_start(g2[t * 128:(t + 1) * 128, 0:48], ob[:])

        _edge_layer(nc, tc, ctx, tab=g1tab, tabw=G1W, elem=G1E,
                    nlow_tab=LOWN, KLO=KLO, KHI=KHI,
                    idx_lo=ilot, idx_hi=ihit, ldst=ldst,
                    rnglo=rlot, rnghi=rhit, adt=adt, adw=8,
                    flush_fn=flush, heads=8, ex_slot=256)
    nc.compile()
    return nc


def build_k3(KLO, KHI):
    KT = KLO + KHI
    nc = bacc.Bacc("TRN2", target_bir_lowering=False, debug=False,
                   num_swdge_queues=4)
    g2tab = nc.dram_tensor("g2tab", [NP, G2W], bf16, kind="ExternalInput")
    ad2 = nc.dram_tensor("ad2", [128, TPC], bf16, kind="ExternalInput")
    ilo = nc.dram_tensor("ilo", [128, TPC * KLO * 8], i16, kind="ExternalInput")
    ihi = nc.dram_tensor("ihi", [128, TPC * KHI * 8], i16, kind="ExternalInput")
    lds = nc.dram_tensor("lds", [128, TPC * KT], f32, kind="ExternalInput")
    rlo = nc.dram_tensor("rlo", [128, TPC * KLO * 2], f32, kind="ExternalInput")
    rhi = nc.dram_tensor("rhi", [128, TPC * KHI * 2], f32, kind="ExternalInput")
    b2r = nc.dram_tensor("b2r", [128, 40], f32, kind="ExternalInput")
    out = nc.dram_tensor("out", [TPC * 128, 40], f32, kind="ExternalOutput")

    with tile.TileContext(nc) as tc, ExitStack() as ctx:
        cp = ctx.enter_context(tc.tile_pool(name="cp", bufs=1))
        fsb = ctx.enter_context(tc.tile_pool(name="fsb", bufs=2))

        adt = cp.tile([128, TPC], bf16)
        nc.sync.dma_start(adt[:], ad2[:, :])
        ilot = cp.tile([128, TPC * KLO * 8], i16)
        nc.sync.dma_start(ilot[:], ilo[:, :])
        ihit = cp.tile([128, TPC * KHI * 8], i16)
        nc.sync.dma_start(ihit[:], ihi[:, :])
        ldst = cp.tile([128, TPC * KT], f32)
        nc.sync.dma_start(ldst[:], lds[:, :])
        rlot = cp.tile([128, TPC * KLO * 2], f32)
        nc.sync.dma_start(rlot[:], rlo[:, :])
        rhit = cp.tile([128, TPC * KHI * 2], f32)
        nc.sync.dma_start(rhit[:], rhi[:, :])
        b2t = cp.tile([128, 40], f32)
        nc.sync.dma_start(b2t[:], b2r[:, :])

        def flush(t, agg):
            rec = fsb.tile([128, 1], f32, tag="rec")
            nc.vector.tensor_scalar_add(rec[:], agg[:, 40:41], 1e-16)
            nc.vector.reciprocal(rec[:], rec[:])
            ov = fsb.tile([128, 40], f32, tag="ov")
            nc.vector.tensor_scalar_mul(ov[:], agg[:, 0:40], rec[:])
            nc.vector.tensor_tensor(out=ov[:], in0=ov[:], in1=b2t[:], op=AOT.add)
            nc.sync.dma_start(out[t * 128:(t + 1) * 128, :], ov[:])

        _edge_layer(nc, tc, ctx, tab=g2tab, tabw=G2W, elem=G2E,
                    nlow_tab=LOWN, KLO=KLO, KHI=KHI,
                    idx_lo=ilot, idx_hi=ihit, ldst=ldst,
                    rnglo=rlot, rnghi=rhit, adt=adt, adw=1,
                    flush_fn=flush, heads=1, ex_slot=41)
    nc.compile()
    return nc


# ------------------------------------------------------------------ runner --
class _CK:
    def __init__(self, nc, n_cores=8):
        import jax
        from jax.sharding import Mesh, PartitionSpec, NamedSharding
        from jax.experimental.shard_map import shard_map
        from concourse.bass2jax import (_bass_exec_p, install_neuronx_cc_hook,
                                        partition_id_tensor)
        install_neuronx_cc_hook()
        self.jax = jax
        self.n_cores = n_cores
        in_names, out_names, out_avals, zero_outs = [], [], [], []
        pname = nc.partition_id_tensor.name if nc.partition_id_tensor else None
        for alloc in nc.m.functions[0].allocations:
            if not isinstance(alloc, mybir.MemoryLocationSet):
                continue
            name = alloc.memorylocations[0].name
            if alloc.kind == "ExternalInput":
                if name != pname:
                    in_names.append(name)
            elif alloc.kind == "ExternalOutput":
                shape = tuple(alloc.tensor_shape)
                dtype = mybir.dt.np(alloc.dtype)
                out_names.append(name)
                out_avals.append(jax.core.ShapedArray(shape, dtype))
                zero_outs.append(np.zeros(shape, dtype))
        self.n_params = len(in_names)
        self.in_names = in_names + out_names + ([pname] if pname else [])
        self.out_names, self.out_avals, self.zero_outs = out_names, out_avals, zero_outs

        def _body(*args):
            ops = list(args)
            if pname is not None:
                ops.append(partition_id_tensor())
            return tuple(_bass_exec_p.bind(
                *ops, out_avals=tuple(out_avals), in_names=tuple(self.in_names),
                out_names=tuple(out_names), lowering_input_output_aliases=(),
                sim_require_finite=True, sim_require_nnan=True, nc=nc))

        devices = jax.devices()[:n_cores]
        mesh = Mesh(np.asarray(devices), ("core",))
        spec = PartitionSpec("core")
        nall = self.n_params + len(out_names)
        self.sharding = NamedSharding(mesh, spec)
        self.fn = jax.jit(shard_map(_body, mesh=mesh, in_specs=(spec,) * nall,
                                    out_specs=(spec,) * len(out_names),
                                    check_rep=False), keep_unused=True)

    def run(self, in_maps):
        jax = self.jax
        concat = [np.concatenate([np.asarray(in_maps[c][n]) for c in range(self.n_cores)], axis=0)
                  for n in self.in_names[:self.n_params]]
        zeros = [np.zeros((self.n_cores * z.shape[0], *z.shape[1:]), z.dtype)
                 for z in self.zero_outs]
        args = [jax.device_put(a, self.sharding) for a in concat + zeros]
        outs = self.fn(*args)
        jax.block_until_ready(outs)
        return [{n: np.asarray(outs[i]).reshape(self.n_cores, *self.out_avals[i].shape)[c]
                 for i, n in enumerate(self.out_names)} for c in range(self.n_cores)]


_CACHE = {}


def kernel(x, edge_index, W1, a_src1, a_dst1, b1, W2, a_src2, a_dst2, b2):
    import time as _time
    _t = [_time.time()]
    def _lap(tag):
        if "KTIME" in __import__("os").environ:
            now = _time.time()
            print(f"  [ktime] {tag}: {now - _t[0]:.3f}s")
            _t[0] = now
    x = np.asarray(x); edge_index = np.asarray(edge_index)
    W1 = np.asarray(W1, np.float32); W2 = np.asarray(W2, np.float32)
    a_src1 = np.asarray(a_src1, np.float32); a_dst1 = np.asarray(a_dst1, np.float32)
    a_src2 = np.asarray(a_src2, np.float32); a_dst2 = np.asarray(a_dst2, np.float32)
    b1 = np.asarray(b1, np.float32); b2 = np.asarray(b2, np.float32)

    key = hash(edge_index.tobytes())
    if key not in _CACHE:
        _CACHE[key] = _pack(edge_index)
    pk = _CACHE[key]
    _lap('pack')
    KLO, KHI, KT = pk["KLO"], pk["KHI"], pk["KT"]

    if "k1" not in _CACHE:
        _CACHE["k1"] = _CK(build_k1())
    if ("k2", KLO, KHI) not in _CACHE:
        _CACHE[("k2", KLO, KHI)] = _CK(build_k2(KLO, KHI))
        _CACHE[("k3", KLO, KHI)] = _CK(build_k3(KLO, KHI))
    k1, k2, k3 = _CACHE["k1"], _CACHE[("k2", KLO, KHI)], _CACHE[("k3", KLO, KHI)]
    _lap('build')

    inv = pk["inv"]
    # permuted x (padded), transposed per core
    xp = np.zeros((NP, IN_DIM), np.float32)
    xp[pk["perm"][:N_NODES]] = x.astype(np.float32)
    xp_bf = xp.astype(ml_dtypes.bfloat16)

    # folded projection weights
    vs1 = np.einsum("khf,hf->kh", W1.reshape(IN_DIM, HEADS, FPH), a_src1)
    vd1 = np.einsum("khf,hf->kh", W1.reshape(IN_DIM, HEADS, FPH), a_dst1)
    Wx1 = np.concatenate([W1, vs1, vd1], axis=1).astype(ml_dtypes.bfloat16)

    in1 = []
    for c in range(NCORES):
        rows = np.concatenate([np.arange(t * 128, (t + 1) * 128)
                               for t in range(c, NT, NCORES)])
        in1.append(dict(xT=np.ascontiguousarray(xp_bf[rows].T), W=Wx1))
    _lap('prep1')
    r1 = k1.run(in1)
    _lap('k1.run')

    # relay 1: assemble full g1 table + per-core ad1 tiles
    g1full = np.zeros((NP, G1W), ml_dtypes.bfloat16)
    for c in range(NCORES):
        rows = np.concatenate([np.arange(t * 128, (t + 1) * 128)
                               for t in range(c, NT, NCORES)])
        g1full[rows] = r1[c]["g1"]

    idx_lo_w = _wrap_idx(pk["idx_lo"])
    idx_hi_w = _wrap_idx(pk["idx_hi"])
    rlo = _clip_runs(pk["run_lo"], KLO)   # [C, T, K, 128, 2]
    rhi = _clip_runs(pk["run_hi"], KHI)

    vs2 = (W2 @ a_src2[0]).astype(np.float32)
    vd2 = (W2 @ a_dst2[0]).astype(np.float32)
    Wx2 = np.zeros((HIDDEN, 48), np.float32)
    Wx2[:, 0:40] = W2
    Wx2[:, 41] = vs2
    Wx2[:, 42] = vd2
    Wx2 = Wx2.astype(ml_dtypes.bfloat16)

    in2 = []
    for c in range(NCORES):
        rows = np.concatenate([np.arange(t * 128, (t + 1) * 128)
                               for t in range(c, NT, NCORES)])
        ad1c = g1full[rows][:, 264:272].reshape(TPC, 128, 8)
        in2.append(dict(
            g1tab=g1full,
            ad1=np.ascontiguousarray(ad1c.transpose(1, 0, 2).reshape(128, TPC * 8)),
            ilo=np.ascontiguousarray(idx_lo_w[c].transpose(1, 0, 2).reshape(128, -1)),
            ihi=np.ascontiguousarray(idx_hi_w[c].transpose(1, 0, 2).reshape(128, -1)),
            lds=np.ascontiguousarray(
                pk["ldst"][c].reshape(TPC, KT, 128).transpose(2, 0, 1)
                .reshape(128, -1).astype(np.float32)),
            rlo=np.ascontiguousarray(rlo[c].transpose(2, 0, 1, 3).reshape(128, -1)),
            rhi=np.ascontiguousarray(rhi[c].transpose(2, 0, 1, 3).reshape(128, -1)),
            b1r=np.tile(b1[None, :], (128, 1)),
            W2=Wx2,
        ))
    _lap('prep2')
    r2 = k2.run(in2)
    _lap('k2.run')

    # relay 2: g2 table + ad2 tiles
    g2full = np.zeros((NP, G2W), ml_dtypes.bfloat16)
    for c in range(NCORES):
        rows = np.concatenate([np.arange(t * 128, (t + 1) * 128)
                               for t in range(c, NT, NCORES)])
        g2full[rows, 0:48] = r2[c]["g2"][:, 0:48]

    in3 = []
    for c in range(NCORES):
        rows = np.concatenate([np.arange(t * 128, (t + 1) * 128)
                               for t in range(c, NT, NCORES)])
        ad2c = g2full[rows][:, 42].reshape(TPC, 128)
        m2 = in2[c]
        in3.append(dict(
            g2tab=g2full, ad2=np.ascontiguousarray(ad2c.T),
            ilo=m2["ilo"], ihi=m2["ihi"], lds=m2["lds"],
            rlo=m2["rlo"], rhi=m2["rhi"],
            b2r=np.tile(b2[None, :], (128, 1)),
        ))
    _lap('prep3')
    r3 = k3.run(in3)
    _lap('k3.run')

    outp = np.zeros((NP, 40), np.float32)
    for c in range(NCORES):
        rows = np.concatenate([np.arange(t * 128, (t + 1) * 128)
                               for t in range(c, NT, NCORES)])
        outp[rows] = r3[c]["out"]
    _lap('unshard')
    return outp[pk["perm"][:N_NODES]].astype(np.float32)

